# revision 4
# baseline (speedup 1.0000x reference)
"""TRN2 Bass kernel for the LSQ-quantized 2-layer MLP.

reference computation:
    wq1 = lsq_quant(w1, alpha1); wq2 = lsq_quant(w2, alpha2)   (tiny 256x256)
    h = relu(x @ wq1.T + b1)
    y = sigmoid(h @ wq2.T + b2)                                 x: [262144, 256] f32

Data-parallel over 8 NeuronCores (32768 tokens/core). Host precomputes the
integer LSQ levels k = round(clip(w/a, -8, 7)) (exact in bf16) and the
effective scale a; the device matmuls run in bf16 on integer weights with
fp32 PSUM accumulation, and the scale a is folded into the activation
(relu(a*z + b), sigmoid(a*z + b)), so the only precision loss is the bf16
rounding of x and h.

Per 512-token macro-tile on each core:
    SWDGE cast-DMA x (f32 DRAM -> bf16 SBUF)
    -> 8x DMA-transpose (SBUF->SBUF, xbar) -> xT
    -> fc1 matmuls (bf16, w1 chunks stationary) -> hT in PSUM (f32)
    -> relu+scale on DVE (or ACT when b1 != 0) -> bf16 SBUF
    -> fc2 matmuls (bf16, hT chunks stationary) -> y in PSUM, natural layout
    -> sigmoid+scale on ACT -> f32 SBUF -> DMA out
"""

import ml_dtypes
import numpy as np

import concourse.bass as bass
import concourse.mybir as mybir
import concourse.tile as tile
from concourse import bacc
from concourse.bass import ts
from concourse.bass_utils import run_bass_kernel_spmd

N_CORES = 8
N_TOK = 262144
C = 256
TOK_PER_CORE = N_TOK // N_CORES  # 32768
T_MACRO = 512
N_MACROS = TOK_PER_CORE // T_MACRO  # 64
P = 128

F32 = mybir.dt.float32
BF16 = mybir.dt.bfloat16

_program_cache = {}


def _build_program(use_b1: bool, use_b2: bool):
    nc = bacc.Bacc("TRN2", target_bir_lowering=False, debug=False, num_devices=N_CORES)

    x_d = nc.declare_dram_parameter("x", [TOK_PER_CORE, C], F32, isOutput=False)
    w1k_d = nc.declare_dram_parameter("w1k", [P, 2, C], BF16, isOutput=False)
    w2k_d = nc.declare_dram_parameter("w2k", [P, 2, C], BF16, isOutput=False)
    a1_d = nc.declare_dram_parameter("a1", [P, 1], F32, isOutput=False)
    a2_d = nc.declare_dram_parameter("a2", [P, 1], F32, isOutput=False)
    if use_b1:
        b1s_d = nc.declare_dram_parameter("b1s", [P, 2], F32, isOutput=False)
    if use_b2:
        b2bc_d = nc.declare_dram_parameter("b2bc", [P, 512], F32, isOutput=False)
    y_d = nc.declare_dram_parameter("y", [TOK_PER_CORE, C], F32, isOutput=True)

    # token index = m*512 + g*128 + p
    x_v = x_d.rearrange("(m g p) c -> m p g c", g=4, p=P)
    y_v = y_d.rearrange("(m g p) c -> m p g c", g=4, p=P)

    with tile.TileContext(nc) as tc:
        with (
            tc.tile_pool(name="const", bufs=1) as const_pool,
            tc.tile_pool(name="sb_x", bufs=4) as sb_x,
            tc.tile_pool(name="sb_xt", bufs=4) as sb_xt,
            tc.tile_pool(name="sb_ht", bufs=4) as sb_ht,
            tc.tile_pool(name="sb_y", bufs=4) as sb_y,
            tc.tile_pool(name="ps_h", bufs=3, space="PSUM") as ps_h,
            tc.tile_pool(name="ps_y", bufs=3, space="PSUM") as ps_y,
        ):
            w1k = const_pool.tile([P, 2, C], BF16)
            w2k = const_pool.tile([P, 2, C], BF16)
            nc.sync.dma_start(w1k[:], w1k_d[:])
            nc.sync.dma_start(w2k[:], w2k_d[:])
            a1 = const_pool.tile([P, 1], F32)
            a2 = const_pool.tile([P, 1], F32)
            nc.sync.dma_start(a1[:], a1_d[:])
            nc.sync.dma_start(a2[:], a2_d[:])
            if use_b1:
                b1s = const_pool.tile([P, 2], F32)
                nc.sync.dma_start(b1s[:], b1s_d[:])
            if use_b2:
                b2bc = const_pool.tile([P, 512], F32)
                nc.sync.dma_start(b2bc[:], b2bc_d[:])

            for m in range(N_MACROS):
                # f32 DRAM -> bf16 SBUF cast during the DMA (SWDGE)
                x_sb = sb_x.tile([P, 4, C], BF16, tag="x")
                nc.gpsimd.dma_start(x_sb[:], x_v[m])

                # xbar DMA transpose: xt[ci, co, g*128+p] = x[g*128+p, co*128+ci]
                xt = sb_xt.tile([P, 2, T_MACRO], BF16, tag="xt")
                for co in range(2):
                    for g in range(4):
                        nc.sync.dma_start(
                            xt[:, co, ts(g, P)],
                            x_sb[:, g, ts(co, P)],
                            transpose=True,
                        )

                # fc1: hT[j_chunk] = sum_c w1k[:,c,jchunk].T @ xT[:,c,:]
                ht = sb_ht.tile([P, 2, T_MACRO], BF16, tag="ht")
                for j in range(2):
                    pht = ps_h.tile([P, T_MACRO], F32, tag="pht")
                    for c in range(2):
                        nc.tensor.matmul(
                            pht[:],
                            w1k[:, c, ts(j, P)],
                            xt[:, c, :],
                            start=(c == 0),
                            stop=(c == 1),
                        )
                    if use_b1:
                        nc.scalar.activation(
                            ht[:, j, :],
                            pht[:],
                            mybir.ActivationFunctionType.Relu,
                            bias=b1s[:, j : j + 1],
                            scale=a1[:],
                        )
                    else:
                        # relu(a1*z) on DVE: (z * a1) max 0
                        nc.vector.tensor_scalar(
                            ht[:, j, :],
                            pht[:],
                            a1[:],
                            0.0,
                            mybir.AluOpType.mult,
                            mybir.AluOpType.max,
                        )

                # fc2: y[tok_chunk] = sum_c ht[:,c,tokchunk].T @ w2k[:,c,:]
                y_sb = sb_y.tile([P, 4, C], F32, tag="y")
                for half in range(2):
                    py = ps_y.tile([P, 512], F32, tag="py")
                    for tg in range(2):
                        t = half * 2 + tg
                        for c in range(2):
                            nc.tensor.matmul(
                                py[:, ts(tg, C)],
                                ht[:, c, ts(t, P)],
                                w2k[:, c, :],
                                start=(c == 0),
                                stop=(c == 1),
                            )
                    if use_b2:
                        nc.vector.tensor_scalar(
                            py[:], py[:], a2[:], None, mybir.AluOpType.mult
                        )
                        nc.vector.tensor_add(py[:], py[:], b2bc[:])
                        nc.scalar.activation(
                            y_sb[:, half * 2 : half * 2 + 2, :],
                            py[:].rearrange("p (a b) -> p a b", a=2),
                            mybir.ActivationFunctionType.Sigmoid,
                        )
                    else:
                        nc.scalar.activation(
                            y_sb[:, half * 2 : half * 2 + 2, :],
                            py[:].rearrange("p (a b) -> p a b", a=2),
                            mybir.ActivationFunctionType.Sigmoid,
                            scale=a2[:],
                        )
                nc.scalar.dma_start(y_v[m], y_sb[:])

    nc.compile()
    return nc


def _quantize_lsq_int(w: np.ndarray, alpha) -> tuple[np.ndarray, np.float32]:
    """Integer LSQ levels k = round(clip(w/a, -8, 7)) and effective scale a,
    replicating the reference forward numerics in np float32."""
    one = np.float32(1.0)
    g = one / np.sqrt(np.float32(w.size * 7))
    alpha = np.float32(alpha)
    a = np.float32(alpha * g) + np.float32(alpha * np.float32(one - g))
    t = np.clip((w / a).astype(np.float32), np.float32(-8.0), np.float32(7.0))
    r = (np.round(t) - t).astype(np.float32)
    q = (t + r).astype(np.float32)  # integer levels in [-8, 7]
    return q, a


def _prepare(x, w1, b1, alpha1, w2, b2, alpha2):
    x = np.ascontiguousarray(np.asarray(x, dtype=np.float32))
    w1 = np.asarray(w1, dtype=np.float32)
    w2 = np.asarray(w2, dtype=np.float32)
    b1 = np.asarray(b1, dtype=np.float32)
    b2 = np.asarray(b2, dtype=np.float32)

    k1, a1 = _quantize_lsq_int(w1, alpha1)
    k2, a2 = _quantize_lsq_int(w2, alpha2)

    # fc1 lhsT layout: w1k[ci, co, j] = k1[j, co*128+ci]
    w1k = np.ascontiguousarray(k1.T.reshape(2, P, C).transpose(1, 0, 2)).astype(
        ml_dtypes.bfloat16
    )
    # fc2 moving operand: w2k[ci, co, j] = k2[j, co*128+ci]
    w2k = np.ascontiguousarray(k2.T.reshape(2, P, C).transpose(1, 0, 2)).astype(
        ml_dtypes.bfloat16
    )

    use_b1 = bool(np.any(b1))
    use_b2 = bool(np.any(b2))
    key = (use_b1, use_b2)
    if key not in _program_cache:
        _program_cache[key] = _build_program(use_b1, use_b2)
    nc = _program_cache[key]

    a1_col = np.full((P, 1), a1, dtype=np.float32)
    a2_col = np.full((P, 1), a2, dtype=np.float32)

    shards = np.split(x, N_CORES, axis=0)
    in_maps = []
    for s in shards:
        m = {
            "x": np.ascontiguousarray(s),
            "w1k": w1k,
            "w2k": w2k,
            "a1": a1_col,
            "a2": a2_col,
        }
        if use_b1:
            m["b1s"] = np.ascontiguousarray(b1.reshape(2, P).T)
        if use_b2:
            m["b2bc"] = np.ascontiguousarray(
                np.broadcast_to(np.concatenate([b2, b2]), (P, 512))
            )
        in_maps.append(m)
    return nc, in_maps


def kernel(x, w1, b1, alpha1, w2, b2, alpha2):
    nc, in_maps = _prepare(x, w1, b1, alpha1, w2, b2, alpha2)
    res = run_bass_kernel_spmd(nc, in_maps, list(range(N_CORES)))
    out = np.concatenate([res.results[i]["y"] for i in range(N_CORES)], axis=0)
    return out


# revision 5
# speedup vs baseline: 5.2643x; 5.2643x over previous
"""TRN2 Bass kernel for the LSQ-quantized 2-layer MLP.

reference computation:
    wq1 = lsq_quant(w1, alpha1); wq2 = lsq_quant(w2, alpha2)   (tiny 256x256)
    h = relu(x @ wq1.T + b1)
    y = sigmoid(h @ wq2.T + b2)                                 x: [262144, 256] f32

Data-parallel over 8 NeuronCores (32768 tokens/core). The host quantizes the
weights (256x256 elementwise, replicated) and lays x out channel-major per
shard, so the contraction dim lands on SBUF partitions with plain DMAs and
the device does no transposes at all. Matmuls run as float32r (TF32-like,
1 cycle/row on the PE vs 4 for fp32, ~1e-4 relative error) with fp32 PSUM
accumulation.

Per 512-token macro-tile on each core:
    DMA xT tile (f32r)
    -> fc1 matmuls (w1 chunks stationary) -> hT in PSUM (f32)
    -> relu(+b1) on DVE -> f32r SBUF
    -> fc2 matmuls (hT chunks stationary) -> y in PSUM, natural [tok, ch]
    -> sigmoid on ACT -> f32 SBUF -> DMA out (ACT queue, no head-of-line
       blocking of the next load on the sync queue)
"""

import numpy as np

import concourse.bass as bass
import concourse.mybir as mybir
import concourse.tile as tile
from concourse import bacc
from concourse.bass import ts
from concourse.bass_utils import run_bass_kernel_spmd

N_CORES = 8
N_TOK = 262144
C = 256
TOK_PER_CORE = N_TOK // N_CORES  # 32768
T_MACRO = 512
N_MACROS = TOK_PER_CORE // T_MACRO  # 64
P = 128

F32 = mybir.dt.float32
F32R = mybir.dt.float32r

_program_cache = {}


def _build_program(use_b1: bool, use_b2: bool):
    nc = bacc.Bacc("TRN2", target_bir_lowering=False, debug=False, num_devices=N_CORES)

    xt_d = nc.declare_dram_parameter("xt", [C, TOK_PER_CORE], F32R, isOutput=False)
    w1t_d = nc.declare_dram_parameter("w1t", [P, 2, C], F32R, isOutput=False)
    w2t_d = nc.declare_dram_parameter("w2t", [P, 2, C], F32R, isOutput=False)
    if use_b1:
        b1s_d = nc.declare_dram_parameter("b1s", [P, 2], F32, isOutput=False)
    if use_b2:
        b2bc_d = nc.declare_dram_parameter("b2bc", [P, 512], F32, isOutput=False)
    y_d = nc.declare_dram_parameter("y", [TOK_PER_CORE, C], F32, isOutput=True)

    xt_v = xt_d.rearrange("(co ci) (m t) -> m ci co t", ci=P, t=T_MACRO)
    # token index = m*512 + g*128 + p
    y_v = y_d.rearrange("(m g p) c -> m p g c", g=4, p=P)

    with tile.TileContext(nc) as tc:
        with (
            tc.tile_pool(name="const", bufs=1) as const_pool,
            tc.tile_pool(name="sb_xt", bufs=4) as sb_xt,
            tc.tile_pool(name="sb_ht", bufs=4) as sb_ht,
            tc.tile_pool(name="sb_y", bufs=4) as sb_y,
            tc.tile_pool(name="ps_h", bufs=4, space="PSUM") as ps_h,
            tc.tile_pool(name="ps_y", bufs=4, space="PSUM") as ps_y,
        ):
            w1t = const_pool.tile([P, 2, C], F32R)
            w2t = const_pool.tile([P, 2, C], F32R)
            nc.sync.dma_start(w1t[:], w1t_d[:])
            nc.sync.dma_start(w2t[:], w2t_d[:])
            if use_b1:
                b1s = const_pool.tile([P, 2], F32)
                nc.sync.dma_start(b1s[:], b1s_d[:])
            if use_b2:
                b2bc = const_pool.tile([P, 512], F32)
                nc.sync.dma_start(b2bc[:], b2bc_d[:])

            for m in range(N_MACROS):
                xt = sb_xt.tile([P, 2, T_MACRO], F32R, tag="xt")
                nc.sync.dma_start(xt[:], xt_v[m])

                # fc1: hT[j_chunk] = sum_c w1t[:,c,jchunk].T @ xT[:,c,:]
                ht = sb_ht.tile([P, 2, T_MACRO], F32R, tag="ht")
                for j in range(2):
                    pht = ps_h.tile([P, T_MACRO], F32, tag="pht")
                    for c in range(2):
                        nc.tensor.matmul(
                            pht[:],
                            w1t[:, c, ts(j, P)],
                            xt[:, c, :],
                            start=(c == 0),
                            stop=(c == 1),
                        )
                    if use_b1:
                        # relu(z + b1) on DVE in one op: max(z + b1, 0)
                        nc.vector.tensor_scalar(
                            ht[:, j, :],
                            pht[:],
                            b1s[:, j : j + 1],
                            0.0,
                            mybir.AluOpType.add,
                            mybir.AluOpType.max,
                        )
                    else:
                        nc.vector.tensor_scalar(
                            ht[:, j, :],
                            pht[:],
                            0.0,
                            None,
                            mybir.AluOpType.max,
                        )

                # fc2: y[tok_chunk] = sum_c ht[:,c,tokchunk].T @ w2t[:,c,:]
                y_sb = sb_y.tile([P, 4, C], F32, tag="y")
                for half in range(2):
                    py = ps_y.tile([P, 512], F32, tag="py")
                    for tg in range(2):
                        t = half * 2 + tg
                        for c in range(2):
                            nc.tensor.matmul(
                                py[:, ts(tg, C)],
                                ht[:, c, ts(t, P)],
                                w2t[:, c, :],
                                start=(c == 0),
                                stop=(c == 1),
                            )
                    if use_b2:
                        nc.vector.tensor_add(py[:], py[:], b2bc[:])
                    nc.scalar.activation(
                        y_sb[:, half * 2 : half * 2 + 2, :],
                        py[:].rearrange("p (a b) -> p a b", a=2),
                        mybir.ActivationFunctionType.Sigmoid,
                    )
                nc.scalar.dma_start(y_v[m], y_sb[:])

    nc.compile()
    return nc


def _quantize_lsq(w: np.ndarray, alpha) -> np.ndarray:
    """Replicates reference lsq_quant_weight forward numerics in np float32."""
    one = np.float32(1.0)
    g = one / np.sqrt(np.float32(w.size * 7))
    alpha = np.float32(alpha)
    a = np.float32(alpha * g) + np.float32(alpha * np.float32(one - g))
    t = np.clip((w / a).astype(np.float32), np.float32(-8.0), np.float32(7.0))
    r = (np.round(t) - t).astype(np.float32)
    q = (t + r).astype(np.float32)
    return (q * a).astype(np.float32)


def _prepare(x, w1, b1, alpha1, w2, b2, alpha2):
    x = np.asarray(x, dtype=np.float32)
    w1 = np.asarray(w1, dtype=np.float32)
    w2 = np.asarray(w2, dtype=np.float32)
    b1 = np.asarray(b1, dtype=np.float32)
    b2 = np.asarray(b2, dtype=np.float32)

    wq1 = _quantize_lsq(w1, alpha1)
    wq2 = _quantize_lsq(w2, alpha2)

    # fc1 lhsT layout: w1t[ci, co, j] = wq1[j, co*128+ci]
    w1t = np.ascontiguousarray(wq1.T.reshape(2, P, C).transpose(1, 0, 2))
    # fc2 moving operand: w2t[ci, co, j] = wq2[j, co*128+ci]
    w2t = np.ascontiguousarray(wq2.T.reshape(2, P, C).transpose(1, 0, 2))

    use_b1 = bool(np.any(b1))
    use_b2 = bool(np.any(b2))
    key = (use_b1, use_b2)
    if key not in _program_cache:
        _program_cache[key] = _build_program(use_b1, use_b2)
    nc = _program_cache[key]

    in_maps = []
    for i in range(N_CORES):
        shard = x[i * TOK_PER_CORE : (i + 1) * TOK_PER_CORE]
        m = {
            "xt": np.ascontiguousarray(shard.T),
            "w1t": w1t,
            "w2t": w2t,
        }
        if use_b1:
            m["b1s"] = np.ascontiguousarray(b1.reshape(2, P).T)
        if use_b2:
            m["b2bc"] = np.ascontiguousarray(
                np.broadcast_to(np.concatenate([b2, b2]), (P, 512))
            )
        in_maps.append(m)
    return nc, in_maps


def kernel(x, w1, b1, alpha1, w2, b2, alpha2):
    nc, in_maps = _prepare(x, w1, b1, alpha1, w2, b2, alpha2)
    res = run_bass_kernel_spmd(nc, in_maps, list(range(N_CORES)))
    out = np.concatenate([res.results[i]["y"] for i in range(N_CORES)], axis=0)
    return out
